# revision 9
# baseline (speedup 1.0000x reference)
"""Trainium2 Bass kernel for nn_Memory (GRU-style scan over 16384 rows, d=512).

Collective-free halo scheme: the gate recurrence is a contraction (state
influence decays below 1e-4 within ~40 rows), so each of the 8 cores solves
its own 2048-row block plus a 64-row halo copied from the previous block,
starting from zero state; core 0's halo is zero-filled x rows (which keep the
state exactly 0). No inter-core communication at all.

Per core, a Picard fixed point (11 passes): all gates are computed from the
previous pass's shifted states with batched matmuls, then states are
re-propagated exactly with tensor_tensor_scan (fp32 state). sigma-only
activations (h = 2*sig(2p)-1, the *2-1 on DVE) allow z|h interleaved PSUM
tiles -> fewer, larger ACT instructions. Passes 1..7 use fp8e4m3 U with
DoubleRow matmuls (2 K-tiles per instruction at 0.5 cyc/col); az is always
injected in fp16 via an identity matmul, so fp8 error is confined to the
m@U term and washed out by the final 3 fp16-U passes (verified ~4.7e-3 rel
on the reference inputs). Gate pre-acts are pre-scaled x8 (z) / x16 (h)
host-side; ACT applies scale=1/8.

Layout: time is tiled in 704-column thirds ([z|h] interleaved [128,1408]
fp32 PSUM tiles, double buffered; every matmul PSUM write is split at the
512-column bank grid - writes crossing a 2KB bank boundary corrupt the
tail). Elementwise stages run on chunk-pair x half-L units; scans per
(chunk, half) so next-pass matmuls on early thirds only wait for the
matching scan halves. Output states are kept fp16, PE-transposed back to
row-major and upcast to fp32 on the way out.
"""

import sys

sys.path.insert(0, "/opt/trn_rl_repo")

import numpy as np

import concourse.bass as bass
import concourse.mybir as mybir
import concourse.tile as tile
from concourse.bass_utils import run_bass_kernel_spmd

T = 16384
D = 512
NCORE = 8
BC = T // NCORE          # rows owned per core
H = 64                   # halo rows from previous block
L = BC + H               # rows processed per core
LP = L + 1               # shifted-state plane width (col 0 = zero init)
KCH = D // 128           # 4 state chunks (128 features each)
TH = L // 3              # 704-column thirds
NPASS = 11               # total passes (incl. pass 0)
N16 = 2                  # final fp16 passes
RESID = True             # build the fp8 U residual pairs (used by late passes)
ND = 2                   # fp8 passes (counted from the fp16 tail) that apply
                         # the U residual; earlier fp8 passes' extra error is
                         # contracted away by subsequent passes

FP32 = mybir.dt.float32
FP16 = mybir.dt.float16
FP8 = mybir.dt.float8e4
AF = mybir.ActivationFunctionType
ALU = mybir.AluOpType
DR = mybir.MatmulPerfMode.DoubleRow


def _apply_tile_drain_patch():
    """This container's walrus rejects >1 sync-wait on the TileContext exit
    Drain. Split the accumulated end-of-kernel waits into one Drain per
    semaphore."""
    import bass_rust

    def _drain_and_barrier(self, tick_clock, wait_clock):
        drain_inst = self.nc.sync.drain()
        wait_clock.add_sem_waits(
            drain_inst.ins, tile.ScopedClock({None: tick_clock.global_clock})
        )
        si = drain_inst.ins.sync_info
        if si is not None and len(si.on_wait) > 1:
            waits = list(si.on_wait)
            si.on_wait = waits[:1]
            for w in waits[1:]:
                d2 = self.nc.sync.drain()
                s2 = d2.ins.sync_info
                if s2 is None:
                    d2.ins.sync_info = bass_rust.SyncInfo(on_wait=[w], on_update=[])
                else:
                    s2.on_wait = [w]
        self.nc.all_engine_barrier()
        assert self.sems is not None
        popped = self.nc._tile_sem_poison_stack.pop()
        assert popped is self._sem_poison
        self.nc.clear_and_free_semaphores(list(self.sems.allocated().values()))
        self.nc.all_engine_barrier()

    tile.TileContext._drain_and_barrier = _drain_and_barrier


def _split_multi_waits(nc):
    """This walrus build encodes at most ONE sync-wait per hardware
    instruction. Hoist extra waits onto same-engine NoOps placed immediately
    before the owning instruction."""
    import bass_rust

    nid = 0
    for f in nc.m.functions:
        for b in f.blocks:
            out = []
            changed = False
            for ins in b.instructions:
                si = ins.sync_info
                if si is not None and len(si.on_wait) > 1:
                    waits = list(si.on_wait)
                    for w in waits[:-1]:
                        nop = mybir.InstNoOp(name=f"I-waitsplit-{nid}", ins=[], outs=[])
                        nid += 1
                        nop.engine = ins.engine
                        nop.sync_info = bass_rust.SyncInfo(on_wait=[w], on_update=[])
                        out.append(nop)
                    si.on_wait = waits[-1:]
                    changed = True
                out.append(ins)
            if changed:
                b.instructions = out


def _bank_chunks(tile_lo, tile_hi):
    """Split the PSUM-tile column range [tile_lo, tile_hi) at every multiple
    of 512 (fp32 2KB bank edge): matmul PSUM writes must not cross banks."""
    out = []
    o = tile_lo
    while o < tile_hi:
        nxt = min(tile_hi, (o // 512 + 1) * 512)
        out.append((o, nxt - o))
        o = nxt
    return out


def build_kernel(npass=NPASS, n16=N16, resid=RESID):
    _apply_tile_drain_patch()
    nc = bass.Bass("TRN2")

    NPAIR = 4 if resid else 2   # fp8 stationary pairs per gate-chunk

    # xt: x slice transposed [D, L] fp16 (host-prepped, halo included)
    xt = nc.dram_tensor("xt", [D, L], FP16, kind="ExternalInput")
    # wp: [8*Wz | 16*Wh] fp16 [D, 2D]
    wp = nc.dram_tensor("wp", [D, 2 * D], FP16, kind="ExternalInput")
    # up: [8*Uz | 16*Uh] fp16 [D, 2D]
    up = nc.dram_tensor("up", [D, 2 * D], FP16, kind="ExternalInput")
    # su: fp8 stationary pairs [128, 2, 8*NPAIR, 128] flattened
    su = nc.dram_tensor("su", [128, 2 * 8 * NPAIR * 128], FP8, kind="ExternalInput")
    # bp: biases per gate-chunk [128, 8] fp32 (x8 for z, x16 for h)
    bp = nc.dram_tensor("bp", [128, 8], FP32, kind="ExternalInput")
    i16 = nc.dram_tensor("i16", [128, 128], FP16, kind="ExternalInput")
    i32 = nc.dram_tensor("i32", [128, 128], FP32, kind="ExternalInput")
    ys = nc.dram_tensor("ys", [BC, D], FP32, kind="ExternalOutput")

    with tile.TileContext(nc) as tc:
        consts = tc.alloc_tile_pool(name="consts", bufs=1)
        wsb = consts.tile([128, KCH, 2 * D], FP16, tag="wsb")
        usb = consts.tile([128, KCH, 2 * D], FP16, tag="usb")
        su8 = consts.tile([128, 2, 8 * NPAIR, 128], FP8, tag="su8")
        id16 = consts.tile([128, 128], FP16, tag="id16")
        id32 = consts.tile([128, 128], FP32, tag="id32")
        bsb = consts.tile([128, 8], FP32, tag="bsb")
        # SP queue: only what phase 1 needs to start (wsb; xsb follows below).
        # ACT queue: everything needed later (biases first, weights for the
        # fp8/fp16 passes after).
        nc.sync.dma_start(wsb[:], wp.rearrange("(k p) m -> p k m", p=128))
        nc.scalar.dma_start(bsb[:], bp[:])
        nc.scalar.dma_start(id16[:], i16[:])
        nc.scalar.dma_start(
            su8[:], su.rearrange("p (two g f) -> p two g f", two=2, f=128)
        )
        nc.scalar.dma_start(usb[:], up.rearrange("(k p) m -> p k m", p=128))
        nc.scalar.dma_start(id32[:], i32[:])

        # persistent state/gate tiles
        stp = tc.alloc_tile_pool(name="stp", bufs=1)
        azp = stp.tile([128, 8, L], FP16, tag="azp")      # gate pre-acts
        m8 = stp.tile([128, KCH, LP], FP8, tag="m8")      # shifted states fp8
        mx = stp.tile([128, KCH, LP], FP16, tag="mx")     # shifted states fp16
        mf = stp.tile([128, KCH, L], FP16, tag="mf")      # final states
        gt = stp.tile([128, 2, KCH, L], FP16, tag="gt")   # z | sig(2ph) planes
        d0 = stp.tile([128, KCH, L], FP16, tag="d0")
        d1 = stp.tile([128, KCH, L], FP16, tag="d1")

        # zero the shifted-state init column (scans write cols 1..L)
        nc.vector.memset(m8[:, :, 0:1], 0.0)
        nc.vector.memset(mx[:, :, 0:1], 0.0)

        # ---------------- phase 1: az = xT @ [W] + b ----------------
        with (
            tc.tile_pool(name="p1", bufs=1) as p1,
            tc.tile_pool(name="p1ps", bufs=2, space="PSUM") as p1ps,
        ):
            xsb = p1.tile([128, KCH, L], FP16, tag="xsb")
            xr = xt.rearrange("(k p) t -> p k t", p=128)
            nc.sync.dma_start(xsb[:, :, 0 : L // 2], xr[:, :, 0 : L // 2])
            nc.sync.dma_start(xsb[:, :, L // 2 : L], xr[:, :, L // 2 : L])
            for c in range(KCH):
                for j in (c, 4 + c):
                    for hf in range(2):                   # 1056-column halves
                        base = hf * (L // 2)
                        psa = p1ps.tile([128, L // 2], FP32, tag="psa")
                        for o, w in _bank_chunks(0, L // 2):
                            for k in range(KCH):
                                nc.tensor.matmul(
                                    psa[:, o : o + w],
                                    wsb[:, k, j * 128 : (j + 1) * 128],
                                    xsb[:, k, base + o : base + o + w],
                                    start=(k == 0),
                                    stop=(k == KCH - 1),
                                )
                        if j in (2, 3, 6, 7):
                            nc.scalar.activation(
                                azp[:, j, base : base + L // 2],
                                psa[:],
                                AF.Identity,
                                bias=bsb[:, j : j + 1],
                            )
                        else:
                            nc.vector.tensor_scalar(
                                azp[:, j, base : base + L // 2],
                                psa[:],
                                bsb[:, j : j + 1],
                                None,
                                ALU.add,
                            )
                # pass-0 gates for chunk c, fused into phase 1 so the ACT
                # queue does not serialize them behind all az moves
                for g in range(2):
                    nc.scalar.activation(
                        gt[:, g, c, :],
                        azp[:, g * 4 + c, :],
                        AF.Sigmoid,
                        scale=0.125,
                    )

        # ---------------- passes ----------------
        with tc.tile_pool(name="ps2", bufs=2, space="PSUM") as ps2:
            for p in range(npass):
                first = p == 0
                fp16 = p >= npass - n16
                last = p == npass - 1

                # gate matmuls + sigma, third-major so every chunk's early
                # columns are done before any chunk's late columns (pass 0's
                # gates were already emitted inside phase 1)
                for th in range(3 if not first else 0):   # 704-column thirds
                    base = th * TH
                    for c in range(KCH):
                        pg = ps2.tile([128, 2 * TH], FP32, tag="pg")
                        for g in range(2):
                            j = g * 4 + c
                            for to, w in _bank_chunks(g * TH, (g + 1) * TH):
                                out = pg[:, to : to + w]
                                lo = base + (to - g * TH)
                                nc.tensor.matmul(
                                    out,
                                    id16[:],
                                    azp[:, j, lo : lo + w],
                                    start=True,
                                    stop=False,
                                )
                                if fp16:
                                    for k in range(KCH):
                                        nc.tensor.matmul(
                                            out,
                                            usb[:, k, j * 128 : (j + 1) * 128],
                                            mx[:, k, lo : lo + w],
                                            start=False,
                                            stop=(k == KCH - 1),
                                        )
                                else:
                                    np_p = (
                                        NPAIR
                                        if p >= npass - n16 - ND
                                        else 2
                                    )
                                    for q in range(np_p):
                                        nc.tensor.matmul(
                                            out,
                                            su8[:, :, j * NPAIR + q, :],
                                            m8[:, (q % 2) * 2 : (q % 2) * 2 + 2,
                                               lo : lo + w],
                                            start=False,
                                            stop=(q == np_p - 1),
                                            perf_mode=DR,
                                        )
                        nc.scalar.activation(
                            gt[:, :, c, base : base + TH],
                            pg[:],
                            AF.Sigmoid,
                            scale=0.125,
                        )

                # elementwise stages on chunk-PAIRS x half-L units (amortize
                # per-instruction overhead), scans per (chunk, third) so that
                # next-pass matmuls on third t wait only on scans of third t.
                HF = L // 2
                if last:
                    sdst, soff = mf, 0
                elif p >= npass - n16 - 1:
                    sdst, soff = mx, 1
                else:
                    sdst, soff = m8, 1

                for hf in range(2):
                    for pr in (0, 2):
                        sl = slice(hf * HF, (hf + 1) * HF)
                        # h = 2*sig(2ph) - 1, in place on gt h-plane
                        nc.vector.tensor_scalar(
                            gt[:, 1, pr : pr + 2, sl],
                            gt[:, 1, pr : pr + 2, sl],
                            2.0, -1.0, ALU.mult, ALU.add,
                        )
                        d1eng = (
                            nc.vector if (hf == 0 or fp16 or first) else nc.gpsimd
                        )
                        d1eng.tensor_tensor(
                            d1[:, pr : pr + 2, sl],
                            gt[:, 0, pr : pr + 2, sl],
                            gt[:, 1, pr : pr + 2, sl],
                            ALU.mult,
                        )
                        deng = nc.vector
                        deng.tensor_scalar(
                            d0[:, pr : pr + 2, sl],
                            gt[:, 0, pr : pr + 2, sl],
                            -1.0, 1.0, ALU.mult, ALU.add,
                        )
                        for c in (pr, pr + 1):
                            a, b = hf * HF, (hf + 1) * HF
                            init = (
                                0.0
                                if hf == 0
                                else sdst[:, c, a + soff - 1 : a + soff]
                            )
                            nc.vector.tensor_tensor_scan(
                                sdst[:, c, a + soff : b + soff],
                                d0[:, c, a:b],
                                d1[:, c, a:b],
                                init,
                                ALU.mult,
                                ALU.add,
                            )

        # ---------------- output: transpose m32 cols H..L, store ----------
        with (
            tc.tile_pool(name="outs", bufs=8) as outs,
            tc.tile_pool(name="pso", bufs=8, space="PSUM") as pso,
        ):
            for tt in range(BC // 128):
                psy = pso.tile([128, 512], FP16, tag="psy")
                for c in range(KCH):
                    nc.tensor.transpose(
                        psy[:, c * 128 : (c + 1) * 128],
                        mf[:, c, H + tt * 128 : H + (tt + 1) * 128],
                        id16[:],
                    )
                yst = outs.tile([128, D], FP32, tag="yst")
                if tt % 2 == 0:
                    nc.scalar.activation(yst[:], psy[:], AF.Identity)
                    nc.sync.dma_start(ys[tt * 128 : (tt + 1) * 128, :], yst[:])
                else:
                    nc.vector.tensor_copy(yst[:], psy[:])
                    nc.scalar.dma_start(ys[tt * 128 : (tt + 1) * 128, :], yst[:])

        stp.release()
        consts.release()

    _split_multi_waits(nc)
    return nc


def _host_prep(inputs, resid=RESID):
    from ml_dtypes import float8_e4m3fn as f8

    Wz = np.asarray(inputs["Wz"], np.float32)
    Wh = np.asarray(inputs["Wh"], np.float32)
    Uz = np.asarray(inputs["Uz"], np.float32)
    Uh = np.asarray(inputs["Uh"], np.float32)
    bz = np.asarray(inputs["bz"], np.float32)
    bh = np.asarray(inputs["bh"], np.float32)

    wp = np.concatenate([8 * Wz, 16 * Wh], axis=1).astype(np.float16)
    up = np.concatenate([8 * Uz, 16 * Uh], axis=1).astype(np.float16)
    u8 = up.astype(f8)
    v8 = (up.astype(np.float32) - u8.astype(np.float32)).astype(f8)

    npair = 4 if resid else 2
    # su[p, i, j*npair+q, f]: pair-plane i of group q for gate-chunk j.
    # q=0: (U_k0, U_k1), q=1: (U_k2, U_k3); resid adds the V pairs as q=2,3.
    su = np.zeros((128, 2, 8 * npair, 128), dtype=f8)
    for j in range(8):
        for q in range(npair):
            mat = u8 if q < 2 else v8
            qq = q % 2
            for i in range(2):
                k = qq * 2 + i
                su[:, i, j * npair + q, :] = mat[
                    k * 128 : (k + 1) * 128, j * 128 : (j + 1) * 128
                ]
    bpack = np.stack(
        [
            (8 * bz if j < 4 else 16 * bh)[(j % 4) * 128 : (j % 4 + 1) * 128]
            for j in range(8)
        ],
        axis=1,
    ).astype(np.float32)

    return {
        "wp": wp,
        "up": up,
        "su": np.ascontiguousarray(su.reshape(128, -1)),
        "bp": bpack,
        "i16": np.eye(128, dtype=np.float16),
        "i32": np.eye(128, dtype=np.float32),
    }


def _prep_xt(x):
    """Per-core transposed x slices [D, L] fp16, halo from previous block,
    core 0 zero-padded."""
    xf = np.asarray(x, np.float32)
    xpad = np.vstack([np.zeros((H, D), np.float32), xf]).astype(np.float16)
    return [np.ascontiguousarray(xpad[c * BC : c * BC + L].T) for c in range(NCORE)]


_CACHE = {}


def kernel(**inputs: np.ndarray) -> np.ndarray:
    import jax

    common = _host_prep(inputs)
    xts = _prep_xt(inputs["x"])
    dev = [d for d in jax.devices() if d.platform != "cpu"][0]

    if "nc" not in _CACHE:
        _CACHE["nc"] = build_kernel()
    in_maps = [{"xt": xts[c], **common} for c in range(NCORE)]
    last_exc = None
    for attempt in range(5):
        if attempt:
            import time

            time.sleep(2.0 * attempt)
        try:
            with jax.default_device(dev):
                res = run_bass_kernel_spmd(
                    _CACHE["nc"], in_maps, core_ids=list(range(NCORE))
                )
            return np.concatenate(
                [np.asarray(res.results[c]["ys"]) for c in range(NCORE)], axis=0
            )
        except Exception as e:
            last_exc = e
            if "UNRECOVERABLE" not in str(e) and "NRT" not in str(e):
                raise
    raise last_exc


if __name__ == "__main__":
    rng = np.random.RandomState(0)
    ins = {
        "x": rng.randn(T, D).astype(np.float32),
        "Wz": (rng.randn(D, D) / np.sqrt(D)).astype(np.float32),
        "Uz": (rng.randn(D, D) / np.sqrt(D)).astype(np.float32),
        "bz": np.zeros(D, np.float32),
        "Wh": (rng.randn(D, D) / np.sqrt(D)).astype(np.float32),
        "Uh": (rng.randn(D, D) / np.sqrt(D)).astype(np.float32),
        "bh": np.zeros(D, np.float32),
    }
    out = kernel(**ins)
    print("out", out.shape, out.dtype, np.abs(out).max())


# revision 10
# speedup vs baseline: 1.0005x; 1.0005x over previous
"""Trainium2 Bass kernel for nn_Memory (GRU-style scan over 16384 rows, d=512).

Collective-free halo scheme: the gate recurrence is a contraction (state
influence decays below 1e-4 within ~40 rows), so each of the 8 cores solves
its own 2048-row block plus a 64-row halo copied from the previous block,
starting from zero state; core 0's halo is zero-filled x rows (which keep the
state exactly 0). No inter-core communication at all.

Per core, a Picard fixed point (11 passes): all gates are computed from the
previous pass's shifted states with batched matmuls, then states are
re-propagated exactly with tensor_tensor_scan (fp32 state). sigma-only
activations (h = 2*sig(2p)-1, the *2-1 on DVE) allow z|h interleaved PSUM
tiles -> fewer, larger ACT instructions. Passes 1..7 use fp8e4m3 U with
DoubleRow matmuls (2 K-tiles per instruction at 0.5 cyc/col); az is always
injected in fp16 via an identity matmul, so fp8 error is confined to the
m@U term and washed out by the final 3 fp16-U passes (verified ~4.7e-3 rel
on the reference inputs). Gate pre-acts are pre-scaled x8 (z) / x16 (h)
host-side; ACT applies scale=1/8.

Layout: time is tiled in 704-column thirds ([z|h] interleaved [128,1408]
fp32 PSUM tiles, double buffered; every matmul PSUM write is split at the
512-column bank grid - writes crossing a 2KB bank boundary corrupt the
tail). Elementwise stages run on chunk-pair x half-L units; scans per
(chunk, half) so next-pass matmuls on early thirds only wait for the
matching scan halves. Output states are kept fp16, PE-transposed back to
row-major and upcast to fp32 on the way out.
"""

import sys

sys.path.insert(0, "/opt/trn_rl_repo")

import numpy as np

import concourse.bass as bass
import concourse.mybir as mybir
import concourse.tile as tile
from concourse.bass_utils import run_bass_kernel_spmd

T = 16384
D = 512
NCORE = 8
BC = T // NCORE          # rows owned per core
H = 64                   # halo rows from previous block
L = BC + H               # rows processed per core
LP = L + 1               # shifted-state plane width (col 0 = zero init)
KCH = D // 128           # 4 state chunks (128 features each)
TH = L // 3              # 704-column thirds
NPASS = 11               # total passes (incl. pass 0)
N16 = 2                  # final fp16 passes
RESID = True             # build the fp8 U residual pairs (used by late passes)
ND = 2                   # fp8 passes (counted from the fp16 tail) that apply
                         # the U residual; earlier fp8 passes' extra error is
                         # contracted away by subsequent passes

FP32 = mybir.dt.float32
FP16 = mybir.dt.float16
FP8 = mybir.dt.float8e4
AF = mybir.ActivationFunctionType
ALU = mybir.AluOpType
DR = mybir.MatmulPerfMode.DoubleRow


def _apply_tile_drain_patch():
    """This container's walrus rejects >1 sync-wait on the TileContext exit
    Drain. Split the accumulated end-of-kernel waits into one Drain per
    semaphore."""
    import bass_rust

    def _drain_and_barrier(self, tick_clock, wait_clock):
        drain_inst = self.nc.sync.drain()
        wait_clock.add_sem_waits(
            drain_inst.ins, tile.ScopedClock({None: tick_clock.global_clock})
        )
        si = drain_inst.ins.sync_info
        if si is not None and len(si.on_wait) > 1:
            waits = list(si.on_wait)
            si.on_wait = waits[:1]
            for w in waits[1:]:
                d2 = self.nc.sync.drain()
                s2 = d2.ins.sync_info
                if s2 is None:
                    d2.ins.sync_info = bass_rust.SyncInfo(on_wait=[w], on_update=[])
                else:
                    s2.on_wait = [w]
        self.nc.all_engine_barrier()
        assert self.sems is not None
        popped = self.nc._tile_sem_poison_stack.pop()
        assert popped is self._sem_poison
        self.nc.clear_and_free_semaphores(list(self.sems.allocated().values()))
        self.nc.all_engine_barrier()

    tile.TileContext._drain_and_barrier = _drain_and_barrier


def _split_multi_waits(nc):
    """This walrus build encodes at most ONE sync-wait per hardware
    instruction. Hoist extra waits onto same-engine NoOps placed immediately
    before the owning instruction."""
    import bass_rust

    nid = 0
    for f in nc.m.functions:
        for b in f.blocks:
            out = []
            changed = False
            for ins in b.instructions:
                si = ins.sync_info
                if si is not None and len(si.on_wait) > 1:
                    waits = list(si.on_wait)
                    for w in waits[:-1]:
                        nop = mybir.InstNoOp(name=f"I-waitsplit-{nid}", ins=[], outs=[])
                        nid += 1
                        nop.engine = ins.engine
                        nop.sync_info = bass_rust.SyncInfo(on_wait=[w], on_update=[])
                        out.append(nop)
                    si.on_wait = waits[-1:]
                    changed = True
                out.append(ins)
            if changed:
                b.instructions = out


def _bank_chunks(tile_lo, tile_hi):
    """Split the PSUM-tile column range [tile_lo, tile_hi) at every multiple
    of 512 (fp32 2KB bank edge): matmul PSUM writes must not cross banks."""
    out = []
    o = tile_lo
    while o < tile_hi:
        nxt = min(tile_hi, (o // 512 + 1) * 512)
        out.append((o, nxt - o))
        o = nxt
    return out


def build_kernel(npass=NPASS, n16=N16, resid=RESID):
    _apply_tile_drain_patch()
    nc = bass.Bass("TRN2")

    NPAIR = 4 if resid else 2   # fp8 stationary pairs per gate-chunk

    # xt: x slice transposed [D, L] fp16 (host-prepped, halo included)
    xt = nc.dram_tensor("xt", [D, L], FP16, kind="ExternalInput")
    # wp: [8*Wz | 16*Wh] fp16 [D, 2D]
    wp = nc.dram_tensor("wp", [D, 2 * D], FP16, kind="ExternalInput")
    # up: [8*Uz | 16*Uh] fp16 [D, 2D]
    up = nc.dram_tensor("up", [D, 2 * D], FP16, kind="ExternalInput")
    # su: fp8 stationary pairs [128, 2, 8*NPAIR, 128] flattened
    su = nc.dram_tensor("su", [128, 2 * 8 * NPAIR * 128], FP8, kind="ExternalInput")
    # bp: biases per gate-chunk [128, 8] fp32 (x8 for z, x16 for h)
    bp = nc.dram_tensor("bp", [128, 8], FP32, kind="ExternalInput")
    i16 = nc.dram_tensor("i16", [128, 128], FP16, kind="ExternalInput")
    i32 = nc.dram_tensor("i32", [128, 128], FP32, kind="ExternalInput")
    ys = nc.dram_tensor("ys", [BC, D], FP32, kind="ExternalOutput")

    with tile.TileContext(nc) as tc:
        consts = tc.alloc_tile_pool(name="consts", bufs=1)
        wsb = consts.tile([128, KCH, 2 * D], FP16, tag="wsb")
        usb = consts.tile([128, KCH, 2 * D], FP16, tag="usb")
        su8 = consts.tile([128, 2, 8 * NPAIR, 128], FP8, tag="su8")
        id16 = consts.tile([128, 128], FP16, tag="id16")
        id32 = consts.tile([128, 128], FP32, tag="id32")
        bsb = consts.tile([128, 8], FP32, tag="bsb")
        # SP queue: only what phase 1 needs to start (wsb; xsb follows below).
        # ACT queue: everything needed later (biases first, weights for the
        # fp8/fp16 passes after).
        nc.sync.dma_start(wsb[:], wp.rearrange("(k p) m -> p k m", p=128))
        nc.scalar.dma_start(bsb[:], bp[:])
        nc.scalar.dma_start(id16[:], i16[:])
        nc.scalar.dma_start(
            su8[:], su.rearrange("p (two g f) -> p two g f", two=2, f=128)
        )
        nc.scalar.dma_start(usb[:], up.rearrange("(k p) m -> p k m", p=128))
        nc.scalar.dma_start(id32[:], i32[:])

        # persistent state/gate tiles
        stp = tc.alloc_tile_pool(name="stp", bufs=1)
        azp = stp.tile([128, 8, L], FP16, tag="azp")      # gate pre-acts
        m8 = stp.tile([128, KCH, LP], FP8, tag="m8")      # shifted states fp8
        mx = stp.tile([128, KCH, LP], FP16, tag="mx")     # shifted states fp16
        mf = stp.tile([128, KCH, L], FP16, tag="mf")      # final states
        gt = stp.tile([128, 2, KCH, L], FP16, tag="gt")   # z | sig(2ph) planes
        d0 = stp.tile([128, KCH, L], FP16, tag="d0")
        d1 = stp.tile([128, KCH, L], FP16, tag="d1")

        # zero the shifted-state init column (scans write cols 1..L)
        nc.vector.memset(m8[:, :, 0:1], 0.0)
        nc.vector.memset(mx[:, :, 0:1], 0.0)

        # ---------------- phase 1: az = xT @ [W] + b ----------------
        with (
            tc.tile_pool(name="p1", bufs=1) as p1,
            tc.tile_pool(name="p1ps", bufs=2, space="PSUM") as p1ps,
        ):
            xsb = p1.tile([128, KCH, L], FP16, tag="xsb")
            xr = xt.rearrange("(k p) t -> p k t", p=128)
            nc.sync.dma_start(xsb[:, :, 0 : L // 2], xr[:, :, 0 : L // 2])
            nc.sync.dma_start(xsb[:, :, L // 2 : L], xr[:, :, L // 2 : L])
            for c in range(KCH):
                for j in (c, 4 + c):
                    for hf in range(2):                   # 1056-column halves
                        base = hf * (L // 2)
                        psa = p1ps.tile([128, L // 2], FP32, tag="psa")
                        for o, w in _bank_chunks(0, L // 2):
                            for k in range(KCH):
                                nc.tensor.matmul(
                                    psa[:, o : o + w],
                                    wsb[:, k, j * 128 : (j + 1) * 128],
                                    xsb[:, k, base + o : base + o + w],
                                    start=(k == 0),
                                    stop=(k == KCH - 1),
                                )
                        if j in (2, 3, 6, 7):
                            nc.scalar.activation(
                                azp[:, j, base : base + L // 2],
                                psa[:],
                                AF.Identity,
                                bias=bsb[:, j : j + 1],
                            )
                        else:
                            nc.vector.tensor_scalar(
                                azp[:, j, base : base + L // 2],
                                psa[:],
                                bsb[:, j : j + 1],
                                None,
                                ALU.add,
                            )
                # pass-0 gates for chunk c, fused into phase 1 so the ACT
                # queue does not serialize them behind all az moves
                for g in range(2):
                    nc.scalar.activation(
                        gt[:, g, c, :],
                        azp[:, g * 4 + c, :],
                        AF.Sigmoid,
                        scale=0.125,
                    )

        # ---------------- passes ----------------
        with tc.tile_pool(name="ps2", bufs=2, space="PSUM") as ps2:
            for p in range(npass):
                first = p == 0
                fp16 = p >= npass - n16
                last = p == npass - 1

                # gate matmuls + sigma, third-major so every chunk's early
                # columns are done before any chunk's late columns (pass 0's
                # gates were already emitted inside phase 1)
                for th in range(3 if not first else 0):   # 704-column thirds
                    base = th * TH
                    for c in range(KCH):
                        pg = ps2.tile([128, 2 * TH], FP32, tag="pg")
                        for g in range(2):
                            j = g * 4 + c
                            for to, w in _bank_chunks(g * TH, (g + 1) * TH):
                                out = pg[:, to : to + w]
                                lo = base + (to - g * TH)
                                nc.tensor.matmul(
                                    out,
                                    id16[:],
                                    azp[:, j, lo : lo + w],
                                    start=True,
                                    stop=False,
                                )
                                if fp16:
                                    for k in range(KCH):
                                        nc.tensor.matmul(
                                            out,
                                            usb[:, k, j * 128 : (j + 1) * 128],
                                            mx[:, k, lo : lo + w],
                                            start=False,
                                            stop=(k == KCH - 1),
                                        )
                                else:
                                    np_p = (
                                        NPAIR
                                        if p >= npass - n16 - ND
                                        else 2
                                    )
                                    for q in range(np_p):
                                        nc.tensor.matmul(
                                            out,
                                            su8[:, :, j * NPAIR + q, :],
                                            m8[:, (q % 2) * 2 : (q % 2) * 2 + 2,
                                               lo : lo + w],
                                            start=False,
                                            stop=(q == np_p - 1),
                                            perf_mode=DR,
                                        )
                        nc.scalar.activation(
                            gt[:, :, c, base : base + TH],
                            pg[:],
                            AF.Sigmoid,
                            scale=0.125,
                        )

                # elementwise stages on chunk-PAIRS x half-L units (amortize
                # per-instruction overhead), scans per (chunk, third) so that
                # next-pass matmuls on third t wait only on scans of third t.
                HF = L // 2
                if last:
                    sdst, soff = mf, 0
                elif p >= npass - n16 - 1:
                    sdst, soff = mx, 1
                else:
                    sdst, soff = m8, 1

                for hf in range(2):
                    for pr in (0, 2):
                        sl = slice(hf * HF, (hf + 1) * HF)
                        # h = 2*sig(2ph) - 1, in place on gt h-plane
                        nc.vector.tensor_scalar(
                            gt[:, 1, pr : pr + 2, sl],
                            gt[:, 1, pr : pr + 2, sl],
                            2.0, -1.0, ALU.mult, ALU.add,
                        )
                        d1eng = (
                            nc.vector if (hf == 0 or fp16 or first) else nc.gpsimd
                        )
                        d1eng.tensor_tensor(
                            d1[:, pr : pr + 2, sl],
                            gt[:, 0, pr : pr + 2, sl],
                            gt[:, 1, pr : pr + 2, sl],
                            ALU.mult,
                        )
                        deng = nc.vector
                        deng.tensor_scalar(
                            d0[:, pr : pr + 2, sl],
                            gt[:, 0, pr : pr + 2, sl],
                            -1.0, 1.0, ALU.mult, ALU.add,
                        )
                        for c in (pr, pr + 1):
                            a, b = hf * HF, (hf + 1) * HF
                            init = (
                                0.0
                                if hf == 0
                                else sdst[:, c, a + soff - 1 : a + soff]
                            )
                            nc.vector.tensor_tensor_scan(
                                sdst[:, c, a + soff : b + soff],
                                d0[:, c, a:b],
                                d1[:, c, a:b],
                                init,
                                ALU.mult,
                                ALU.add,
                            )

        # ---------------- output: transpose m32 cols H..L, store ----------
        with (
            tc.tile_pool(name="outs", bufs=12) as outs,
            tc.tile_pool(name="pso", bufs=8, space="PSUM") as pso,
        ):
            for tt in range(BC // 128):
                psy = pso.tile([128, 512], FP16, tag="psy")
                for c in range(KCH):
                    nc.tensor.transpose(
                        psy[:, c * 128 : (c + 1) * 128],
                        mf[:, c, H + tt * 128 : H + (tt + 1) * 128],
                        id16[:],
                    )
                yst = outs.tile([128, D], FP32, tag="yst")
                if tt % 2 == 0:
                    nc.scalar.activation(yst[:], psy[:], AF.Identity)
                    nc.sync.dma_start(ys[tt * 128 : (tt + 1) * 128, :], yst[:])
                else:
                    nc.vector.tensor_copy(yst[:], psy[:])
                    nc.scalar.dma_start(ys[tt * 128 : (tt + 1) * 128, :], yst[:])

        stp.release()
        consts.release()

    _split_multi_waits(nc)
    return nc


def _host_prep(inputs, resid=RESID):
    from ml_dtypes import float8_e4m3fn as f8

    Wz = np.asarray(inputs["Wz"], np.float32)
    Wh = np.asarray(inputs["Wh"], np.float32)
    Uz = np.asarray(inputs["Uz"], np.float32)
    Uh = np.asarray(inputs["Uh"], np.float32)
    bz = np.asarray(inputs["bz"], np.float32)
    bh = np.asarray(inputs["bh"], np.float32)

    wp = np.concatenate([8 * Wz, 16 * Wh], axis=1).astype(np.float16)
    up = np.concatenate([8 * Uz, 16 * Uh], axis=1).astype(np.float16)
    u8 = up.astype(f8)
    v8 = (up.astype(np.float32) - u8.astype(np.float32)).astype(f8)

    npair = 4 if resid else 2
    # su[p, i, j*npair+q, f]: pair-plane i of group q for gate-chunk j.
    # q=0: (U_k0, U_k1), q=1: (U_k2, U_k3); resid adds the V pairs as q=2,3.
    su = np.zeros((128, 2, 8 * npair, 128), dtype=f8)
    for j in range(8):
        for q in range(npair):
            mat = u8 if q < 2 else v8
            qq = q % 2
            for i in range(2):
                k = qq * 2 + i
                su[:, i, j * npair + q, :] = mat[
                    k * 128 : (k + 1) * 128, j * 128 : (j + 1) * 128
                ]
    bpack = np.stack(
        [
            (8 * bz if j < 4 else 16 * bh)[(j % 4) * 128 : (j % 4 + 1) * 128]
            for j in range(8)
        ],
        axis=1,
    ).astype(np.float32)

    return {
        "wp": wp,
        "up": up,
        "su": np.ascontiguousarray(su.reshape(128, -1)),
        "bp": bpack,
        "i16": np.eye(128, dtype=np.float16),
        "i32": np.eye(128, dtype=np.float32),
    }


def _prep_xt(x):
    """Per-core transposed x slices [D, L] fp16, halo from previous block,
    core 0 zero-padded."""
    xf = np.asarray(x, np.float32)
    xpad = np.vstack([np.zeros((H, D), np.float32), xf]).astype(np.float16)
    return [np.ascontiguousarray(xpad[c * BC : c * BC + L].T) for c in range(NCORE)]


_CACHE = {}


def kernel(**inputs: np.ndarray) -> np.ndarray:
    import jax

    common = _host_prep(inputs)
    xts = _prep_xt(inputs["x"])
    dev = [d for d in jax.devices() if d.platform != "cpu"][0]

    if "nc" not in _CACHE:
        _CACHE["nc"] = build_kernel()
    in_maps = [{"xt": xts[c], **common} for c in range(NCORE)]
    last_exc = None
    for attempt in range(5):
        if attempt:
            import time

            time.sleep(2.0 * attempt)
        try:
            with jax.default_device(dev):
                res = run_bass_kernel_spmd(
                    _CACHE["nc"], in_maps, core_ids=list(range(NCORE))
                )
            return np.concatenate(
                [np.asarray(res.results[c]["ys"]) for c in range(NCORE)], axis=0
            )
        except Exception as e:
            last_exc = e
            if "UNRECOVERABLE" not in str(e) and "NRT" not in str(e):
                raise
    raise last_exc


if __name__ == "__main__":
    rng = np.random.RandomState(0)
    ins = {
        "x": rng.randn(T, D).astype(np.float32),
        "Wz": (rng.randn(D, D) / np.sqrt(D)).astype(np.float32),
        "Uz": (rng.randn(D, D) / np.sqrt(D)).astype(np.float32),
        "bz": np.zeros(D, np.float32),
        "Wh": (rng.randn(D, D) / np.sqrt(D)).astype(np.float32),
        "Uh": (rng.randn(D, D) / np.sqrt(D)).astype(np.float32),
        "bh": np.zeros(D, np.float32),
    }
    out = kernel(**ins)
    print("out", out.shape, out.dtype, np.abs(out).max())
